# revision 1
# baseline (speedup 1.0000x reference)
"""Trainium2 Bass kernel for a 2-layer LSTM + Dense head.

Model (per reference):
  L1: LSTM(H1=32, tanh),  L2: LSTM(H2=16, relu), Dense(12) on last h2.
  x: [512, 512, 64] f32.

Strategy: pure data parallelism, batch 512 -> 64 per core over 8 cores.
Per core, both layers are merged into shared engine ops by stacking their
hidden rows on partitions: rows [h1(0:32) | h2(32:48) | ones(48)].
Gates are laid out along the free dim in blocks [g|i|f|o] x 64(batch), so the
whole cell update for BOTH layers is:
  - 4 input-proj matmuls (K=65 incl. a ones-row for b1, M=48 zero-padded,
    off the critical path; opens the PSUM bank) + 4 merged recurrent
    matmuls ([U1;0 | W2;U2;b2]^T [h1;h2;ones], K=49, M=48) on-chain
  - tanh(g1) + sigmoid([i|f]) + sigmoid(o) on ACT straight from PSUM,
    relu(g2) on DVE (runs during the ACT ops)
  - one fused TT mul producing [i*g | f*c] for both layers, one TT add -> c
  - tanh(c1) on ACT with relu(c2) on DVE in parallel, one TT mul -> h
x is transposed to [F, batch] per step via off-critical-path PE transposes
(DMA transpose is 2-byte-only on TRN2), batched 8 steps per PSUM->SBUF copy.
Predicted device time (InstructionCostModel): ~1.63 ms; the serial chain is
sem-latency + ACT/PE-bound at ~3.1 us per timestep.
"""

import sys

import numpy as np

if "/opt/trn_rl_repo" not in sys.path:
    sys.path.insert(0, "/opt/trn_rl_repo")

B_FULL = 512
T_FULL = 512
F = 64
H1, H2, OUT = 32, 16, 12
N_CORES = 8
B = B_FULL // N_CORES  # 64 batch per core

L1R0, L1R1 = 0, H1          # L1 rows 0:32
L2R0, L2R1 = H1, H1 + H2    # L2 rows 32:48
NR = H1 + H2                # 48
ONESROW = NR                # row 48 = ones

_NC_CACHE = {}


def build_nc(T=T_FULL, unroll_feed=True):
    import concourse.mybir as mybir
    from concourse import bacc
    from concourse.masks import make_identity
    from concourse.tile import TileContext

    fp32 = mybir.dt.float32
    Sig = mybir.ActivationFunctionType.Sigmoid
    Tanh = mybir.ActivationFunctionType.Tanh
    mult = mybir.AluOpType.mult
    add = mybir.AluOpType.add

    CT = 32 if T >= 32 else T   # x DMA chunk (timesteps)
    LA = 16 if T >= 32 else T   # transpose lookahead
    CPY = 8 if T >= 8 else T    # timesteps per PSUM->SBUF xT copy
    XT_RING = 32 if T >= 32 else T  # xT ring slots

    nc = bacc.Bacc(None, target_bir_lowering=False)

    x_d = nc.dram_tensor("x", [B, T, F], fp32, kind="ExternalInput")
    wA_d = nc.dram_tensor("wA", [F + 1, 4 * NR], fp32, kind="ExternalInput")
    wB_d = nc.dram_tensor("wB", [NR + 1, 4 * NR], fp32, kind="ExternalInput")
    wD_d = nc.dram_tensor("wD", [NR + 1, OUT], fp32, kind="ExternalInput")
    ri_d = nc.dram_tensor("rinit", [NR + 1, B], fp32, kind="ExternalInput")
    out_d = nc.dram_tensor("out", [OUT, B], fp32, kind="ExternalOutput")

    with TileContext(nc) as tc:
        with (
            tc.tile_pool(name="singles", bufs=1) as sp,
            tc.tile_pool(name="xraw", bufs=2) as xrp,
            tc.tile_pool(name="psum_z", bufs=4, space="PSUM") as pz,
            tc.tile_pool(name="psum_t", bufs=2, space="PSUM") as pt,
            tc.tile_pool(name="psum_o", bufs=1, space="PSUM") as po,
        ):
            wA = sp.tile([F + 1, 4 * NR], fp32)
            wB = sp.tile([NR + 1, 4 * NR], fp32)
            wD = sp.tile([NR + 1, OUT], fp32)
            nc.sync.dma_start(wA[:], wA_d[:])
            nc.sync.dma_start(wB[:], wB_d[:])
            nc.sync.dma_start(wD[:], wD_d[:])

            ident = sp.tile([64, 64], fp32)
            make_identity(nc, ident[:])

            # recurrent state [h1(0:32); h2(32:48); ones(48)] x batch, x2 (ping/pong)
            rhsA = sp.tile([NR + 1, B], fp32)
            rhsB = sp.tile([NR + 1, B], fp32)
            rhs = [rhsA, rhsB]
            for r in rhs:  # zeros + ones row 48 (compute ops can't start at p48)
                nc.sync.dma_start(r[:], ri_d[:])

            GC = sp.tile([NR, 2 * B], fp32)  # cols [g' | c]
            nc.gpsimd.memset(GC[:], 0.0)
            S = sp.tile([NR, 4 * B], fp32)   # sigma(z) blocks [g|i|f|o]
            M = sp.tile([NR, 2 * B], fp32)   # [i*g | f*c]
            TH = sp.tile([NR, B], fp32)      # [tanh(c1); relu(c2)]

            xT = sp.tile([F + 1, XT_RING * B], fp32)  # transposed x ring + ones row
            nc.gpsimd.memset(xT[F : F + 1, :], 1.0)

            state = {"xraw": None, "psumT": None}

            def feed(k):
                t = k + LA
                if t >= T or t < 0:
                    return
                if t % CT == 0:
                    state["xraw"] = xrp.tile([B, CT * F], fp32, tag="xraw", name="xraw")
                    nc.sync.dma_start(state["xraw"][:], x_d[:, t : t + CT, :])
                if t % CPY == 0:
                    state["psumT"] = pt.tile([F, CPY * B], fp32, tag="psumT", name="psumT")
                j = t % CT
                nc.tensor.transpose(
                    state["psumT"][:, (t % CPY) * B : (t % CPY + 1) * B],
                    state["xraw"][:, j * F : (j + 1) * F],
                    ident[:],
                )
                if t % CPY == CPY - 1:
                    base = (t - (CPY - 1)) % XT_RING
                    nc.scalar.copy(
                        xT[0:F, base * B : (base + CPY) * B], state["psumT"][:]
                    )

            for k in range(-LA, 0):
                feed(k)

            for k in range(T + 1):
                feed(k)
                r_cur = rhs[k % 2]
                r_nxt = rhs[(k + 1) % 2]
                last = k == T
                # active rows for the merged elementwise ops:
                # k=0 -> L1 only (L2 state must stay zero until its first
                # real step at k=1), k=T -> L2 only (epilogue), else both.
                if k == 0:
                    ra, rb = 0, H1
                elif last:
                    ra, rb = L2R0, L2R1
                else:
                    ra, rb = 0, NR
                z = pz.tile([NR, 4 * B], fp32, tag="z", name="z")

                # PSUM zero regions are 2KB (the whole bank row), so the
                # first matmul starts the group and the last one stops it.
                # mmA (input proj, cols 32:48 zero-padded) opens rows 0:48 off
                # the critical path; the merged recurrent matmul does
                # [U1;0 | W2;U2;b2]^T [h1;h2;ones] for one gate in ONE op.
                if not last:
                    rk = k % XT_RING
                    for j in range(4):
                        nc.tensor.matmul(
                            z[0:NR, j * B : (j + 1) * B],
                            wA[:, j * NR : (j + 1) * NR],
                            xT[:, rk * B : (rk + 1) * B],
                            start=(j == 0),
                            stop=False,
                        )
                for j in range(4):
                    nc.tensor.matmul(
                        z[0:NR, j * B : (j + 1) * B],
                        wB[:, j * NR : (j + 1) * NR],
                        r_cur[0 : NR + 1, :],
                        start=(j == 0 and last),
                        stop=(j == 3),
                    )

                zl2 = k > 0              # L2 z rows valid this iter
                if zl2:  # relu(z_g2) straight from PSUM, early on DVE
                    nc.vector.tensor_scalar_max(
                        GC[L2R0:L2R1, 0:B], z[L2R0:L2R1, 0:B], 0.0
                    )
                if not last:  # tanh(g1) straight from PSUM (same ACT table set)
                    nc.scalar.activation(GC[L1R0:L1R1, 0:B], z[L1R0:L1R1, 0:B], Tanh)
                # sigmoid over [i|f] blocks (one op), then the o block
                # separately: keeps the op feeding TTmul as short as possible
                # (a merged [i|f|o] op measured +27us total on the chain)
                nc.scalar.activation(S[ra:rb, B : 3 * B], z[ra:rb, B : 3 * B], Sig)
                nc.scalar.activation(
                    S[ra:rb, 3 * B : 4 * B], z[ra:rb, 3 * B : 4 * B], Sig
                )
                # c update: [i*g | f*c] then add
                nc.vector.tensor_mul(
                    M[ra:rb, :], S[ra:rb, B : 3 * B], GC[ra:rb, :]
                )
                nc.vector.tensor_add(
                    GC[ra:rb, B : 2 * B], M[ra:rb, 0:B], M[ra:rb, B : 2 * B]
                )
                if not last:
                    nc.scalar.activation(
                        TH[L1R0:L1R1, :], GC[L1R0:L1R1, B : 2 * B], Tanh
                    )
                if zl2:
                    nc.vector.tensor_scalar_max(
                        TH[L2R0:L2R1, :], GC[L2R0:L2R1, B : 2 * B], 0.0
                    )
                # h = act(c) * sigma(o) -> next-step rhs
                nc.vector.tensor_mul(
                    r_nxt[ra:rb, :], TH[ra:rb, :], S[ra:rb, 3 * B : 4 * B]
                )

            # dense head: [0(h1); Wd(h2); bd]^T [h1; h2; ones]
            r_fin = rhs[(T + 1) % 2]
            opsum = po.tile([OUT, B], fp32, tag="o", name="opsum")
            nc.tensor.matmul(
                opsum[:], wD[:], r_fin[0 : NR + 1, :], start=True, stop=True
            )
            osb = sp.tile([OUT, B], fp32)
            nc.scalar.copy(osb[:], opsum[:])
            nc.sync.dma_start(out_d[:], osb[:])

    nc.compile()
    return nc


def _get_nc(T=T_FULL):
    if T not in _NC_CACHE:
        _NC_CACHE[T] = build_nc(T)
    return _NC_CACHE[T]


def prep_weights(W1, U1, b1, W2, U2, b2, Wd, bd):
    """Pack weights into the 4 lhsT tensors (gate blocks [g,i,f,o])."""

    def gates(w, H):
        w = np.asarray(w, np.float32)
        i, f, g, o = (w[..., k * H : (k + 1) * H] for k in range(4))
        return [g, i, f, o]  # block order

    W1g, b1g = gates(W1, H1), gates(b1, H1)
    W2g, U1g, U2g, b2g = gates(W2, H2), gates(U1, H1), gates(U2, H2), gates(b2, H2)
    # wA[j]: [65, 48] = [[W1_j; b1_j] | zeros]
    wA = np.concatenate(
        [
            np.concatenate(
                [
                    np.concatenate([W1g[j], b1g[j][None, :]], axis=0),
                    np.zeros((F + 1, H2), np.float32),
                ],
                axis=1,
            )
            for j in range(4)
        ],
        axis=1,
    ).astype(np.float32)
    # wB[j]: [49, 48] = [[U1_j; 0] | [W2_j; U2_j; b2_j]]
    wB = np.concatenate(
        [
            np.concatenate(
                [
                    np.concatenate(
                        [U1g[j], np.zeros((H2 + 1, H1), np.float32)], axis=0
                    ),
                    np.concatenate(
                        [W2g[j], U2g[j], b2g[j][None, :]], axis=0
                    ),
                ],
                axis=1,
            )
            for j in range(4)
        ],
        axis=1,
    ).astype(np.float32)
    wD = np.concatenate(
        [
            np.zeros((H1, OUT), np.float32),
            np.asarray(Wd, np.float32),
            np.asarray(bd, np.float32)[None, :],
        ],
        axis=0,
    ).astype(np.float32)
    return wA, wB, wD


def run_cores(nc, x, weights, T, trace=False):
    from concourse.bass_utils import run_bass_kernel_spmd

    weights = dict(weights)
    rinit = np.zeros((NR + 1, B), np.float32)
    rinit[NR, :] = 1.0
    weights["rinit"] = rinit
    x = np.ascontiguousarray(np.asarray(x, np.float32))
    in_maps = [
        dict(x=np.ascontiguousarray(x[c * B : (c + 1) * B, :T]), **weights)
        for c in range(N_CORES)
    ]
    res = run_bass_kernel_spmd(nc, in_maps, core_ids=list(range(N_CORES)), trace=trace)
    out = np.concatenate([r["out"].T for r in res.results], axis=0)
    return out.astype(np.float32), res


def kernel(x, W1, U1, b1, W2, U2, b2, Wd, bd):
    wA, wB, wD = prep_weights(W1, U1, b1, W2, U2, b2, Wd, bd)
    nc = _get_nc(T_FULL)
    out, _ = run_cores(nc, x, dict(wA=wA, wB=wB, wD=wD), T_FULL)
    return out



# revision 3
# speedup vs baseline: 1.6093x; 1.6093x over previous
"""Trainium2 Bass kernel for a 2-layer LSTM + Dense head — v4 (wide hybrid).

Model: L1 LSTM(32, tanh), L2 LSTM(16, relu), Dense(12) on last h2.
x: [512, 512, 64] f32. Data-parallel: batch 512 -> 64 per core, 8 cores.

Layout: both layers merged on partitions (L1 rows 0:32, L2 rows 32:48),
gates along the free dim in blocks [g|i|f|o] x 64 batch — so every
two-tensor DVE op pairs operands at the SAME partition base (a hard
walrus/birverifier requirement for SBUF operands).

Key optimizations over the original baseline:
- Single activation function: sigma(x) = 0.5*tanh(x/2)+0.5 with the /2
  pre-folded into the i,f,o weight columns, so ONE tanh op covers the
  [g|i|f] blocks of BOTH layers (the o block gets its own op, off the
  critical chain: sigma(o) is only needed after the c' tanh).
- The c-state lives in the S tile's o-columns (192:256), so the XY
  product reads [Tg | c] as one strided AP and computes
  [sigma(i)*tau(g) | sigma(f)*c] for L1 in a single tensor_tensor.
- relu(g2) + c2 live in AUX so L2's XY mirrors L1's, and
  h2' = sigma(o2)*relu(c2') is one fused scalar_tensor_tensor (max,mult).
- bf16 matmuls (1 PE cycle/row vs 4 for fp32) and bf16 DVE (2x/4x modes).
- Same-engine sync deps downgraded to queue-order deps (depfix).
Predicted ~1.98us/step vs baseline 3.18us/step.
"""

import sys

import numpy as np

if "/opt/trn_rl_repo" not in sys.path:
    sys.path.insert(0, "/opt/trn_rl_repo")

B_FULL = 512
T_FULL = 512
F = 64
H1, H2, OUT = 32, 16, 12
N_CORES = 8
B = B_FULL // N_CORES  # 64 batch per core

NR = H1 + H2  # 48 state rows [h1 | h2]
GB = NR       # gate block width (cols per gate block)

_NC_CACHE = {}


# Opcodes that execute synchronously on the issuing engine's pipeline.
_ENGINE_OPS = {
    "Matmult", "Activation", "TensorTensor", "TensorScalarPtr",
    "TensorScalar", "TensorReduce", "TensorTensorScan", "Select",
    "Iota", "Memset", "LoadActFuncSet", "Ldweights",
}


def _downgrade_same_engine_syncs(nc):
    """Engines execute their queue in order and interlock their own
    pipelines (the DVE DRAIN, not the semaphore, is the ordering barrier),
    so a same-engine data dependency needs no semaphore wait. Downgrading
    those edges to ordering-only removes ~95ns (DVE) / ~219ns (ACT) of
    serial latency per hop. Cross-engine and DMA edges are untouched."""
    import bass_rust

    DI = bass_rust.DependencyInfo
    inst_map = nc.inst_map
    n = 0
    for inst in list(inst_map.values()):
        if str(inst.opcode) not in _ENGINE_OPS:
            continue
        eng = inst.engine
        if eng is None:
            continue
        for dep_name, info in list(inst.dependency_edges()):
            if not info.sync:
                continue
            dep = inst_map.get(dep_name)
            if dep is None or str(dep.opcode) not in _ENGINE_OPS:
                continue
            if dep.engine != eng:
                continue
            inst.remove_dependency(dep_name)
            inst.add_dependency(dep_name, DI(sync=False, no_sync=True))
            n += 1
    return n



def build_nc(T=T_FULL):
    import concourse.mybir as mybir
    from concourse import bacc
    from concourse.masks import make_identity
    from concourse.tile import TileContext

    fp32 = mybir.dt.float32
    bf16 = mybir.dt.bfloat16
    Tanh = mybir.ActivationFunctionType.Tanh
    add = mybir.AluOpType.add
    mult = mybir.AluOpType.mult
    amax = mybir.AluOpType.max

    CT = 32 if T >= 32 else T   # x DMA chunk (timesteps)
    LA = 16 if T >= 32 else T   # transpose lookahead
    CPY = 8 if T >= 8 else T    # timesteps per PSUM->SBUF xT copy
    XT_RING = 32 if T >= 32 else T

    nc = bacc.Bacc(None, target_bir_lowering=False)

    x_d = nc.dram_tensor("x", [B, T, F], bf16, kind="ExternalInput")
    wA_d = nc.dram_tensor("wA", [F + 1, 4 * NR], bf16, kind="ExternalInput")
    wB_d = nc.dram_tensor("wB", [NR + 1, 4 * NR], bf16, kind="ExternalInput")
    wD_d = nc.dram_tensor("wD", [NR + 1, OUT], bf16, kind="ExternalInput")
    wG_d = nc.dram_tensor("wG", [NR + 1, H2], bf16, kind="ExternalInput")
    ri_d = nc.dram_tensor("rinit", [NR + 1, B], bf16, kind="ExternalInput")
    out_d = nc.dram_tensor("out", [OUT, B], fp32, kind="ExternalOutput")

    with TileContext(nc) as tc:
        with (
            tc.tile_pool(name="singles", bufs=1) as sp,
            tc.tile_pool(name="xraw", bufs=2) as xrp,
            tc.tile_pool(name="psum_z", bufs=2, space="PSUM") as pz,
            tc.tile_pool(name="psum_t", bufs=2, space="PSUM") as pt,
            tc.tile_pool(name="psum_zg", bufs=1, space="PSUM") as pzg,
            tc.tile_pool(name="psum_zo", bufs=1, space="PSUM") as pzo,
        ):
            wA = sp.tile([F + 1, 4 * NR], bf16)
            wB = sp.tile([NR + 1, 4 * NR], bf16)
            wD = sp.tile([NR + 1, OUT], bf16)
            wG = sp.tile([NR + 1, H2], bf16)
            nc.sync.dma_start(wA[:], wA_d[:])
            nc.sync.dma_start(wB[:], wB_d[:])
            nc.sync.dma_start(wD[:], wD_d[:])
            nc.sync.dma_start(wG[:], wG_d[:])

            ident = sp.tile([64, 64], bf16)
            make_identity(nc, ident[:])

            # recurrent state [h1(0:32); h2(32:48); ones(48)], ping-pong
            rA = sp.tile([NR + 1, B], bf16)
            rB = sp.tile([NR + 1, B], bf16)
            rhs = [rA, rB]
            for r in rhs:  # zeros + ones row 48 (compute ops can't start at p48)
                nc.sync.dma_start(r[:], ri_d[:])

            # S cols: [Ti(0:64) | Tf(64:128) | Tg(128:192) | c-state(192:256)]
            # (gate blocks ordered [i|f|g|o] so [Tg | c] is contiguous)
            S = sp.tile([NR, 4 * B], bf16)
            STo = sp.tile([NR, B], bf16)     # tanh(z_o) both layers
            SPo = sp.tile([NR, B], bf16)     # sigma(o) both layers
            SP = sp.tile([NR, 2 * B], bf16)  # [sigma(i) | sigma(f)]
            M = sp.tile([NR, 2 * B], bf16)   # XY products
            # AUX: L2's XY source: [relu(g2)(0:64) | c2-state(64:128)]
            AUX = sp.tile([NR, 2 * B], bf16)
            TC = sp.tile([H1, B], bf16)      # tanh(c1')
            nc.gpsimd.memset(S[:, 3 * B : 4 * B], 0.0)  # c1 = 0
            nc.gpsimd.memset(AUX[:], 0.0)               # c2 = 0

            xT = sp.tile([F + 1, XT_RING * B], bf16)  # x^T ring + ones row
            nc.gpsimd.memset(xT[F : F + 1, :], 1.0)

            state = {"xraw": None, "psumT": None}

            def feed(k):
                t = k + LA
                if t >= T or t < 0:
                    return
                if t % CT == 0:
                    state["xraw"] = xrp.tile([B, CT * F], bf16, tag="xraw", name="xraw")
                    nc.sync.dma_start(state["xraw"][:], x_d[:, t : t + CT, :])
                if t % CPY == 0:
                    state["psumT"] = pt.tile(
                        [F, CPY * B], bf16, tag="psumT", name="psumT"
                    )
                j = t % CT
                nc.tensor.transpose(
                    state["psumT"][:, (t % CPY) * B : (t % CPY + 1) * B],
                    state["xraw"][:, j * F : (j + 1) * F],
                    ident[:],
                )
                if t % CPY == CPY - 1:
                    base = (t - (CPY - 1)) % XT_RING
                    # PSUM->SBUF copy; fills the DVE idle slot at the top of
                    # the step (off the critical chain)
                    nc.vector.tensor_scalar_add(
                        xT[0:F, base * B : (base + CPY) * B], state["psumT"][:], 0.0
                    )

            for k in range(-LA, 0):
                feed(k)

            for k in range(T + 1):
                feed(k)
                r_cur = rhs[k % 2]
                r_nxt = rhs[(k + 1) % 2]
                l1 = k < T    # L1 computes h1_k
                l2 = k > 0    # L2 computes h2_{k-1}

                # Z holds [i|f|g] (the ACT_a chain read); the o block gets
                # its own tile/group so ACT_a's wait covers 3 recurrent
                # matmuls, not 4.
                Z = pz.tile([NR, 3 * B], fp32, tag="z", name="z")
                Zo = pzo.tile([NR, B], fp32, tag="zo", name="zo")
                if l1:
                    rk = k % XT_RING
                    for j in range(3):
                        nc.tensor.matmul(
                            Z[0:NR, j * B : (j + 1) * B],
                            wA[:, j * NR : (j + 1) * NR],
                            xT[:, rk * B : (rk + 1) * B],
                            start=(j == 0),
                            stop=False,
                        )
                    nc.tensor.matmul(
                        Zo[:],
                        wA[:, 3 * NR : 4 * NR],
                        xT[:, rk * B : (rk + 1) * B],
                        start=True,
                        stop=False,
                    )
                for j in range(3):
                    nc.tensor.matmul(
                        Z[0:NR, j * B : (j + 1) * B],
                        wB[:, j * NR : (j + 1) * NR],
                        r_cur[:],
                        start=(j == 0 and not l1),
                        stop=(j == 2),
                    )
                nc.tensor.matmul(
                    Zo[:],
                    wB[:, 3 * NR : 4 * NR],
                    r_cur[:],
                    start=not l1,
                    stop=True,
                )

                if l2:
                    # g2 gets its own matmul + PSUM tile so its DVE relu
                    # does not share a PSUM bank with the ACT readers of Z
                    ZG = pzg.tile([H2, B], fp32, tag="zg", name="zg")
                    nc.tensor.matmul(ZG[:], wG[:], r_cur[:], start=True, stop=True)
                    nc.vector.tensor_scalar_max(AUX[H1:NR, 0:B], ZG[:], 0.0)
                # ACT: tanh over [i|f|g] (the chain op), then the o block
                nc.scalar.activation(S[:, 0 : 3 * B], Z[:], Tanh)
                nc.scalar.activation(STo[:], Zo[:], Tanh)

                # DVE cell phase
                # SP = [sigma(i) | sigma(f)] for both layers (4x ts)
                nc.vector.tensor_scalar(
                    SP[:], S[:, 0 : 2 * B], 0.5, 0.5, mult, add
                )
                if l1:
                    # [sigma(i1)*Tg1 | sigma(f1)*c1] — in1 = [Tg | c],
                    # contiguous cols 128:256
                    nc.vector.tensor_tensor(
                        M[0:H1, :], SP[0:H1, :], S[0:H1, 2 * B : 4 * B], mult
                    )
                    # c1' = i-term + f-term, back into the S tile
                    nc.vector.tensor_tensor(
                        S[0:H1, 3 * B : 4 * B], M[0:H1, 0:B], M[0:H1, B : 2 * B],
                        add,
                    )
                    # tanh(c1')
                    nc.scalar.activation(TC[:], S[0:H1, 3 * B : 4 * B], Tanh)
                # sigma(o) for both layers (off-chain: runs during TC)
                nc.vector.tensor_scalar(
                    SPo[:], STo[:], 0.5, 0.5, mult, add
                )
                if l2:
                    nc.vector.tensor_tensor(
                        M[H1:NR, :], SP[H1:NR, :], AUX[H1:NR, :], mult
                    )
                    nc.vector.tensor_tensor(
                        AUX[H1:NR, B : 2 * B], M[H1:NR, 0:B],
                        M[H1:NR, B : 2 * B], add,
                    )
                    # h2' = sigma(o2) * relu(c2') fused
                    nc.vector.scalar_tensor_tensor(
                        r_nxt[H1:NR, :], AUX[H1:NR, B : 2 * B], 0.0,
                        SPo[H1:NR, :], amax, mult,
                    )
                if l1:
                    # h1' = sigma(o1)*tanh(c1') — last: waits on TC
                    nc.vector.tensor_tensor(
                        r_nxt[0:H1, :], SPo[0:H1, :], TC[:], mult
                    )

            # dense head: [0(h1) | Wd(h2) | bd]^T r_fin
            r_fin = rhs[(T + 1) % 2]
            opsum = pz.tile([OUT, B], fp32, tag="o", name="opsum")
            nc.tensor.matmul(opsum[:], wD[:], r_fin[:], start=True, stop=True)
            osb = sp.tile([OUT, B], fp32)
            nc.scalar.copy(osb[:], opsum[:])
            nc.sync.dma_start(out_d[:], osb[:])

            _downgrade_same_engine_syncs(nc)

    nc.compile()
    return nc


def _get_nc(T=T_FULL):
    if T not in _NC_CACHE:
        _NC_CACHE[T] = build_nc(T)
    return _NC_CACHE[T]


def _bf16(a):
    import ml_dtypes

    return np.asarray(a, np.float32).astype(ml_dtypes.bfloat16)


def prep_weights(W1, U1, b1, W2, U2, b2, Wd, bd):
    """Pack weights: gate blocks [g|i|f|o], sigma blocks (i,f,o) scaled 0.5
    (sigma(z) = 0.5*tanh(z/2)+0.5 with the /2 folded into the weights)."""
    W1, U1, b1 = (np.asarray(a, np.float32) for a in (W1, U1, b1))
    W2, U2, b2 = (np.asarray(a, np.float32) for a in (W2, U2, b2))
    Wd, bd = np.asarray(Wd, np.float32), np.asarray(bd, np.float32)

    def blocks(w, H):  # reference order i,f,g,o -> [i, f, g, o]
        return [w[..., j * H : (j + 1) * H] for j in range(4)]

    cs = [0.5, 0.5, 1.0, 0.5]  # z scale per block [i|f|g|o]

    W1b, b1b = blocks(W1, H1), blocks(b1, H1)
    U1b = blocks(U1, H1)
    W2b, U2b, b2b = blocks(W2, H2), blocks(U2, H2), blocks(b2, H2)

    # wA[j]: [65, 48] = [[W1_j*s; b1_j*s] | zeros(L2)]
    wA = np.concatenate(
        [
            np.concatenate(
                [
                    np.concatenate([W1b[j] * cs[j], b1b[j][None, :] * cs[j]], axis=0),
                    np.zeros((F + 1, H2), np.float32),
                ],
                axis=1,
            )
            for j in range(4)
        ],
        axis=1,
    )
    # wB[j]: [49, 48] = [[U1_j*s; 0] | [W2_j*s; U2_j*s; b2_j*s]]
    wB = np.concatenate(
        [
            np.concatenate(
                [
                    np.concatenate(
                        [U1b[j] * cs[j], np.zeros((H2 + 1, H1), np.float32)], axis=0
                    ),
                    np.concatenate(
                        [W2b[j] * cs[j], U2b[j] * cs[j], b2b[j][None, :] * cs[j]],
                        axis=0,
                    ),
                ],
                axis=1,
            )
            for j in range(4)
        ],
        axis=1,
    )
    wD = np.concatenate(
        [np.zeros((H1, OUT), np.float32), Wd, bd[None, :]], axis=0
    )
    # wG [49, 16]: L2's raw g2 pre-activation [W2_g; U2_g; b2_g]
    wG = np.concatenate(
        [W2b[2], U2b[2], b2b[2][None, :]], axis=0
    )
    return _bf16(wA), _bf16(wB), _bf16(wD), _bf16(wG)


def make_rinit():
    rinit = np.zeros((NR + 1, B), np.float32)
    rinit[NR, :] = 1.0
    return _bf16(rinit)


def run_cores(nc, x, weights, T, trace=False):
    from concourse.bass_utils import run_bass_kernel_spmd

    x = _bf16(x)
    weights = dict(weights, rinit=make_rinit())
    in_maps = [
        dict(x=np.ascontiguousarray(x[c * B : (c + 1) * B, :T]), **weights)
        for c in range(N_CORES)
    ]
    res = run_bass_kernel_spmd(nc, in_maps, core_ids=list(range(N_CORES)), trace=trace)
    out = np.concatenate([np.asarray(r["out"]).T for r in res.results], axis=0)
    return out.astype(np.float32), res


def kernel(x, W1, U1, b1, W2, U2, b2, Wd, bd):
    wA, wB, wD, wG = prep_weights(W1, U1, b1, W2, U2, b2, Wd, bd)
    nc = _get_nc(T_FULL)
    out, _ = run_cores(nc, x, dict(wA=wA, wB=wB, wD=wD, wG=wG), T_FULL)
    return out


# revision 10
# speedup vs baseline: 1.6483x; 1.0242x over previous
"""Trainium2 Bass kernel for a 2-layer LSTM + Dense head — v4 (wide hybrid).

Model: L1 LSTM(32, tanh), L2 LSTM(16, relu), Dense(12) on last h2.
x: [512, 512, 64] f32. Data-parallel: batch 512 -> 64 per core, 8 cores.

Layout: both layers merged on partitions (L1 rows 0:32, L2 rows 32:48),
gates along the free dim in blocks [g|i|f|o] x 64 batch — so every
two-tensor DVE op pairs operands at the SAME partition base (a hard
walrus/birverifier requirement for SBUF operands).

Key optimizations over the original baseline:
- Single activation function: sigma(x) = 0.5*tanh(x/2)+0.5 with the /2
  pre-folded into the i,f,o weight columns, so ONE tanh op covers the
  [g|i|f] blocks of BOTH layers (the o block gets its own op, off the
  critical chain: sigma(o) is only needed after the c' tanh).
- The c-state lives in the S tile's last column block (192:256) right
  after Tg (128:192), so the XY product reads [Tg | c] as one contiguous
  AP and computes [sigma(i)*tau(g) | sigma(f)*c] in a single tensor_tensor.
- relu(g2) + c2 live in AUX so L2's XY mirrors L1's, and
  h2' = sigma(o2)*relu(c2') is one fused scalar_tensor_tensor (max,mult).
- bf16 matmuls (1 PE cycle/row vs 4 for fp32) and bf16 DVE (2x/4x modes).
- Same-engine sync deps downgraded to queue-order deps (depfix).
Measured (TimelineSim, T=512): 1.012ms total vs baseline 1.629ms (1.61x);
rel err 1.04e-2 on the full axon/walrus path.
"""

import sys

import numpy as np

if "/opt/trn_rl_repo" not in sys.path:
    sys.path.insert(0, "/opt/trn_rl_repo")

B_FULL = 512
T_FULL = 512
F = 64
H1, H2, OUT = 32, 16, 12
N_CORES = 8
B = B_FULL // N_CORES  # 64 batch per core

NR = H1 + H2  # 48 state rows [h1 | h2]
GB = NR       # gate block width (cols per gate block)

_NC_CACHE = {}


# Opcodes that execute synchronously on the issuing engine's pipeline.
_ENGINE_OPS = {
    "Matmult", "Activation", "TensorTensor", "TensorScalarPtr",
    "TensorScalar", "TensorReduce", "TensorTensorScan", "Select",
    "Iota", "Memset", "LoadActFuncSet", "Ldweights",
}


def _downgrade_same_engine_syncs(nc):
    """Engines execute their queue in order and interlock their own
    pipelines (the DVE DRAIN, not the semaphore, is the ordering barrier),
    so a same-engine data dependency needs no semaphore wait. Downgrading
    those edges to ordering-only removes ~95ns (DVE) / ~219ns (ACT) of
    serial latency per hop. Cross-engine and DMA edges are untouched."""
    import bass_rust

    DI = bass_rust.DependencyInfo
    inst_map = nc.inst_map
    n = 0
    for inst in list(inst_map.values()):
        if str(inst.opcode) not in _ENGINE_OPS:
            continue
        eng = inst.engine
        if eng is None:
            continue
        for dep_name, info in list(inst.dependency_edges()):
            if not info.sync:
                continue
            dep = inst_map.get(dep_name)
            if dep is None or str(dep.opcode) not in _ENGINE_OPS:
                continue
            if dep.engine != eng:
                continue
            inst.remove_dependency(dep_name)
            inst.add_dependency(dep_name, DI(sync=False, no_sync=True))
            n += 1
    return n



def build_nc(T=T_FULL):
    import concourse.mybir as mybir
    from concourse import bacc
    from concourse.masks import make_identity
    from concourse.tile import TileContext

    fp32 = mybir.dt.float32
    bf16 = mybir.dt.bfloat16
    Tanh = mybir.ActivationFunctionType.Tanh
    add = mybir.AluOpType.add
    mult = mybir.AluOpType.mult
    amax = mybir.AluOpType.max

    CT = 32 if T >= 32 else T   # x DMA chunk (timesteps)
    LA = 16 if T >= 32 else T   # transpose lookahead
    CPY = 8 if T >= 8 else T    # timesteps per PSUM->SBUF xT copy
    XT_RING = 32 if T >= 32 else T

    nc = bacc.Bacc(None, target_bir_lowering=False)

    x_d = nc.dram_tensor("x", [B, T, F], bf16, kind="ExternalInput")
    wA_d = nc.dram_tensor("wA", [F + 1, 4 * NR], bf16, kind="ExternalInput")
    wB_d = nc.dram_tensor("wB", [NR + 1, 4 * NR], bf16, kind="ExternalInput")
    wD_d = nc.dram_tensor("wD", [NR + 1, OUT], bf16, kind="ExternalInput")
    wG_d = nc.dram_tensor("wG", [NR + 1, H2], bf16, kind="ExternalInput")
    ri_d = nc.dram_tensor("rinit", [NR + 1, B], bf16, kind="ExternalInput")
    out_d = nc.dram_tensor("out", [OUT, B], fp32, kind="ExternalOutput")

    with TileContext(nc) as tc:
        with (
            tc.tile_pool(name="singles", bufs=1) as sp,
            tc.tile_pool(name="xraw", bufs=2) as xrp,
            tc.tile_pool(name="psum_z", bufs=2, space="PSUM") as pz,
            tc.tile_pool(name="psum_t", bufs=2, space="PSUM") as pt,
            tc.tile_pool(name="psum_zg", bufs=1, space="PSUM") as pzg,
            tc.tile_pool(name="psum_zo", bufs=1, space="PSUM") as pzo,
        ):
            wA = sp.tile([F + 1, 4 * NR], bf16)
            wB = sp.tile([NR + 1, 4 * NR], bf16)
            wD = sp.tile([NR + 1, OUT], bf16)
            wG = sp.tile([NR + 1, H2], bf16)
            # spread prologue DMAs across sequencers: each dma_start costs
            # ~650ns of dispatch on its issuing engine's sequencer, and the
            # SP queue alone would serialize them ahead of the first steps
            nc.sync.dma_start(wA[:], wA_d[:])
            nc.scalar.dma_start(wB[:], wB_d[:])
            nc.scalar.dma_start(wD[:], wD_d[:])
            nc.gpsimd.dma_start(wG[:], wG_d[:])

            ident = sp.tile([64, 64], bf16)
            make_identity(nc, ident[:])

            # recurrent state [h1(0:32); h2(32:48); ones(48)], ping-pong
            rA = sp.tile([NR + 1, B], bf16)
            rB = sp.tile([NR + 1, B], bf16)
            rhs = [rA, rB]
            # zeros + ones row 48 (compute ops can't start at p48)
            nc.scalar.dma_start(rhs[0][:], ri_d[:])
            nc.gpsimd.dma_start(rhs[1][:], ri_d[:])

            # S cols: [Ti(0:64) | Tf(64:128) | Tg(128:192) | c-state(192:256)]
            # (gate blocks ordered [i|f|g|o] so [Tg | c] is contiguous)
            S = sp.tile([NR, 4 * B], bf16)
            STo = sp.tile([NR, B], bf16)     # tanh(z_o) both layers
            SPo = sp.tile([NR, B], bf16)     # sigma(o) both layers
            SP = sp.tile([NR, 2 * B], bf16)  # [sigma(i) | sigma(f)]
            M = sp.tile([NR, 2 * B], bf16)   # XY products
            # AUX: L2's XY source: [relu(g2)(0:64) | c2-state(64:128)]
            AUX = sp.tile([NR, 2 * B], bf16)
            TC = sp.tile([H1, B], bf16)      # tanh(c1')
            nc.gpsimd.memset(S[:, 3 * B : 4 * B], 0.0)  # c1 = 0
            nc.gpsimd.memset(AUX[:], 0.0)               # c2 = 0

            xT = sp.tile([F + 1, XT_RING * B], bf16)  # x^T ring + ones row
            nc.gpsimd.memset(xT[F : F + 1, :], 1.0)

            state = {"xraw": None, "psumT": None, "last_dve": None,
                     "last_pe": None}

            def _after(inst, anchor):
                # scheduler-only ordering: keep lookahead work out of the
                # critical chain by pinning it after this step's anchor op
                if anchor is not None:
                    import bass_rust

                    inst.ins.add_dependency(
                        anchor.ins.name,
                        bass_rust.DependencyInfo(sync=False, no_sync=True),
                    )

            def feed(k):
                t = k + LA
                if t >= T or t < 0:
                    return
                if t % CT == 0:
                    state["xraw"] = xrp.tile([B, CT * F], bf16, tag="xraw", name="xraw")
                    nc.sync.dma_start(state["xraw"][:], x_d[:, t : t + CT, :])
                if t % CPY == 0:
                    state["psumT"] = pt.tile(
                        [F, CPY * B], bf16, tag="psumT", name="psumT"
                    )
                j = t % CT
                tr = nc.tensor.transpose(
                    state["psumT"][:, (t % CPY) * B : (t % CPY + 1) * B],
                    state["xraw"][:, j * F : (j + 1) * F],
                    ident[:],
                )
                _after(tr, state["last_pe"])
                if t % CPY == CPY - 1:
                    base = (t - (CPY - 1)) % XT_RING
                    # PSUM->SBUF copy; pinned after this step's h' so it runs
                    # in the idle mm+ACT window, never mid-chain
                    cp = nc.vector.tensor_scalar_add(
                        xT[0:F, base * B : (base + CPY) * B], state["psumT"][:], 0.0
                    )
                    _after(cp, state["last_dve"])

            for k in range(-LA, 0):
                feed(k)

            for k in range(T + 1):
                r_cur = rhs[k % 2]
                r_nxt = rhs[(k + 1) % 2]
                l1 = k < T    # L1 computes h1_k
                l2 = k > 0    # L2 computes h2_{k-1}

                # Z holds [i|f|g] (the ACT_a chain read); the o block gets
                # its own tile/group so ACT_a's wait covers 3 recurrent
                # matmuls, not 4.
                Z = pz.tile([NR, 3 * B], fp32, tag="z", name="z")
                Zo = pzo.tile([NR, B], fp32, tag="zo", name="zo")
                if l1:
                    rk = k % XT_RING
                    for j in range(3):
                        nc.tensor.matmul(
                            Z[0:NR, j * B : (j + 1) * B],
                            wA[:, j * NR : (j + 1) * NR],
                            xT[:, rk * B : (rk + 1) * B],
                            start=(j == 0),
                            stop=False,
                        )
                    nc.tensor.matmul(
                        Zo[:],
                        wA[:, 3 * NR : 4 * NR],
                        xT[:, rk * B : (rk + 1) * B],
                        start=True,
                        stop=False,
                    )
                for j in range(3):
                    nc.tensor.matmul(
                        Z[0:NR, j * B : (j + 1) * B],
                        wB[:, j * NR : (j + 1) * NR],
                        r_cur[:],
                        start=(j == 0 and not l1),
                        stop=(j == 2),
                    )
                state["last_pe"] = nc.tensor.matmul(
                    Zo[:],
                    wB[:, 3 * NR : 4 * NR],
                    r_cur[:],
                    start=not l1,
                    stop=True,
                )

                if l2:
                    # g2 gets its own matmul + PSUM tile so its DVE relu
                    # does not share a PSUM bank with the ACT readers of Z
                    ZG = pzg.tile([H2, B], fp32, tag="zg", name="zg")
                    nc.tensor.matmul(ZG[:], wG[:], r_cur[:], start=True, stop=True)
                    nc.vector.tensor_scalar_max(AUX[H1:NR, 0:B], ZG[:], 0.0)
                # ACT: tanh over [i|f|g] (the chain op), then the o block
                nc.scalar.activation(S[:, 0 : 3 * B], Z[:], Tanh)
                nc.scalar.activation(STo[:], Zo[:], Tanh)

                # DVE cell phase
                # SP = [sigma(i) | sigma(f)] for both layers (4x ts)
                nc.vector.tensor_scalar(
                    SP[:], S[:, 0 : 2 * B], 0.5, 0.5, mult, add
                )
                if l1:
                    # [sigma(i1)*Tg1 | sigma(f1)*c1] — in1 = [Tg | c],
                    # contiguous cols 128:256
                    nc.vector.tensor_tensor(
                        M[0:H1, :], SP[0:H1, :], S[0:H1, 2 * B : 4 * B], mult
                    )
                    # c1' = i-term + f-term, back into the S tile
                    nc.vector.tensor_tensor(
                        S[0:H1, 3 * B : 4 * B], M[0:H1, 0:B], M[0:H1, B : 2 * B],
                        add,
                    )
                    # tanh(c1')
                    nc.scalar.activation(TC[:], S[0:H1, 3 * B : 4 * B], Tanh)
                # sigma(o) for both layers (off-chain: runs during TC)
                nc.vector.tensor_scalar(
                    SPo[:], STo[:], 0.5, 0.5, mult, add
                )
                if l2:
                    nc.vector.tensor_tensor(
                        M[H1:NR, :], SP[H1:NR, :], AUX[H1:NR, :], mult
                    )
                    nc.vector.tensor_tensor(
                        AUX[H1:NR, B : 2 * B], M[H1:NR, 0:B],
                        M[H1:NR, B : 2 * B], add,
                    )
                    # h2' = sigma(o2) * relu(c2') fused
                    state["last_dve"] = nc.vector.scalar_tensor_tensor(
                        r_nxt[H1:NR, :], AUX[H1:NR, B : 2 * B], 0.0,
                        SPo[H1:NR, :], amax, mult,
                    )
                if l1:
                    # h1' = sigma(o1)*tanh(c1') — last: waits on TC
                    state["last_dve"] = nc.vector.tensor_tensor(
                        r_nxt[0:H1, :], SPo[0:H1, :], TC[:], mult
                    )
                # feed at the BOTTOM of the iteration: the lookahead
                # transposes/copies then queue after this step's chain ops
                # and run in the idle mm+ACT window of the next step,
                # instead of being scheduler-placed mid-chain
                feed(k)

            # dense head: [0(h1) | Wd(h2) | bd]^T r_fin
            r_fin = rhs[(T + 1) % 2]
            opsum = pz.tile([OUT, B], fp32, tag="o", name="opsum")
            nc.tensor.matmul(opsum[:], wD[:], r_fin[:], start=True, stop=True)
            osb = sp.tile([OUT, B], fp32)
            nc.scalar.copy(osb[:], opsum[:])
            nc.sync.dma_start(out_d[:], osb[:])

            _downgrade_same_engine_syncs(nc)

    nc.compile()
    return nc


def _get_nc(T=T_FULL):
    if T not in _NC_CACHE:
        _NC_CACHE[T] = build_nc(T)
    return _NC_CACHE[T]


def _bf16(a):
    import ml_dtypes

    return np.asarray(a, np.float32).astype(ml_dtypes.bfloat16)


def prep_weights(W1, U1, b1, W2, U2, b2, Wd, bd):
    """Pack weights: gate blocks [g|i|f|o], sigma blocks (i,f,o) scaled 0.5
    (sigma(z) = 0.5*tanh(z/2)+0.5 with the /2 folded into the weights)."""
    W1, U1, b1 = (np.asarray(a, np.float32) for a in (W1, U1, b1))
    W2, U2, b2 = (np.asarray(a, np.float32) for a in (W2, U2, b2))
    Wd, bd = np.asarray(Wd, np.float32), np.asarray(bd, np.float32)

    def blocks(w, H):  # reference order i,f,g,o -> [i, f, g, o]
        return [w[..., j * H : (j + 1) * H] for j in range(4)]

    cs = [0.5, 0.5, 1.0, 0.5]  # z scale per block [i|f|g|o]

    W1b, b1b = blocks(W1, H1), blocks(b1, H1)
    U1b = blocks(U1, H1)
    W2b, U2b, b2b = blocks(W2, H2), blocks(U2, H2), blocks(b2, H2)

    # wA[j]: [65, 48] = [[W1_j*s; b1_j*s] | zeros(L2)]
    wA = np.concatenate(
        [
            np.concatenate(
                [
                    np.concatenate([W1b[j] * cs[j], b1b[j][None, :] * cs[j]], axis=0),
                    np.zeros((F + 1, H2), np.float32),
                ],
                axis=1,
            )
            for j in range(4)
        ],
        axis=1,
    )
    # wB[j]: [49, 48] = [[U1_j*s; 0] | [W2_j*s; U2_j*s; b2_j*s]]
    wB = np.concatenate(
        [
            np.concatenate(
                [
                    np.concatenate(
                        [U1b[j] * cs[j], np.zeros((H2 + 1, H1), np.float32)], axis=0
                    ),
                    np.concatenate(
                        [W2b[j] * cs[j], U2b[j] * cs[j], b2b[j][None, :] * cs[j]],
                        axis=0,
                    ),
                ],
                axis=1,
            )
            for j in range(4)
        ],
        axis=1,
    )
    wD = np.concatenate(
        [np.zeros((H1, OUT), np.float32), Wd, bd[None, :]], axis=0
    )
    # wG [49, 16]: L2's raw g2 pre-activation [W2_g; U2_g; b2_g]
    wG = np.concatenate(
        [W2b[2], U2b[2], b2b[2][None, :]], axis=0
    )
    return _bf16(wA), _bf16(wB), _bf16(wD), _bf16(wG)


def make_rinit():
    rinit = np.zeros((NR + 1, B), np.float32)
    rinit[NR, :] = 1.0
    return _bf16(rinit)


def run_cores(nc, x, weights, T, trace=False):
    from concourse.bass_utils import run_bass_kernel_spmd

    x = _bf16(x)
    weights = dict(weights, rinit=make_rinit())
    in_maps = [
        dict(x=np.ascontiguousarray(x[c * B : (c + 1) * B, :T]), **weights)
        for c in range(N_CORES)
    ]
    res = run_bass_kernel_spmd(nc, in_maps, core_ids=list(range(N_CORES)), trace=trace)
    out = np.concatenate([np.asarray(r["out"]).T for r in res.results], axis=0)
    return out.astype(np.float32), res


def kernel(x, W1, U1, b1, W2, U2, b2, Wd, bd):
    wA, wB, wD, wG = prep_weights(W1, U1, b1, W2, U2, b2, Wd, bd)
    nc = _get_nc(T_FULL)
    out, _ = run_cores(nc, x, dict(wA=wA, wB=wB, wD=wD, wG=wG), T_FULL)
    return out


# revision 12
# speedup vs baseline: 1.6490x; 1.0004x over previous
"""Trainium2 Bass kernel for a 2-layer LSTM + Dense head — v4 (wide hybrid).

Model: L1 LSTM(32, tanh), L2 LSTM(16, relu), Dense(12) on last h2.
x: [512, 512, 64] f32. Data-parallel: batch 512 -> 64 per core, 8 cores.

Layout: both layers merged on partitions (L1 rows 0:32, L2 rows 32:48),
gates along the free dim in blocks [g|i|f|o] x 64 batch — so every
two-tensor DVE op pairs operands at the SAME partition base (a hard
walrus/birverifier requirement for SBUF operands).

Key optimizations over the original baseline:
- Single activation function: sigma(x) = 0.5*tanh(x/2)+0.5 with the /2
  pre-folded into the i,f,o weight columns, so ONE tanh op covers the
  [g|i|f] blocks of BOTH layers (the o block gets its own op, off the
  critical chain: sigma(o) is only needed after the c' tanh).
- The c-state lives in the S tile's last column block (192:256) right
  after Tg (128:192), so the XY product reads [Tg | c] as one contiguous
  AP and computes [sigma(i)*tau(g) | sigma(f)*c] in a single tensor_tensor.
- relu(g2) + c2 live in AUX so L2's XY mirrors L1's, and
  h2' = sigma(o2)*relu(c2') is one fused scalar_tensor_tensor (max,mult).
- bf16 matmuls (1 PE cycle/row vs 4 for fp32) and bf16 DVE (2x/4x modes).
- Same-engine sync deps downgraded to queue-order deps (depfix).
Measured (TimelineSim, T=512): 1.012ms total vs baseline 1.629ms (1.61x);
rel err 1.04e-2 on the full axon/walrus path.
"""

import sys

import numpy as np

if "/opt/trn_rl_repo" not in sys.path:
    sys.path.insert(0, "/opt/trn_rl_repo")

B_FULL = 512
T_FULL = 512
F = 64
H1, H2, OUT = 32, 16, 12
N_CORES = 8
B = B_FULL // N_CORES  # 64 batch per core

NR = H1 + H2  # 48 state rows [h1 | h2]
GB = NR       # gate block width (cols per gate block)

_NC_CACHE = {}


# Opcodes that execute synchronously on the issuing engine's pipeline.
_ENGINE_OPS = {
    "Matmult", "Activation", "TensorTensor", "TensorScalarPtr",
    "TensorScalar", "TensorReduce", "TensorTensorScan", "Select",
    "Iota", "Memset", "LoadActFuncSet", "Ldweights",
}


def _downgrade_same_engine_syncs(nc):
    """Engines execute their queue in order and interlock their own
    pipelines (the DVE DRAIN, not the semaphore, is the ordering barrier),
    so a same-engine data dependency needs no semaphore wait. Downgrading
    those edges to ordering-only removes ~95ns (DVE) / ~219ns (ACT) of
    serial latency per hop. Cross-engine and DMA edges are untouched."""
    import bass_rust

    DI = bass_rust.DependencyInfo
    inst_map = nc.inst_map
    n = 0
    for inst in list(inst_map.values()):
        if str(inst.opcode) not in _ENGINE_OPS:
            continue
        eng = inst.engine
        if eng is None:
            continue
        for dep_name, info in list(inst.dependency_edges()):
            if not info.sync:
                continue
            dep = inst_map.get(dep_name)
            if dep is None or str(dep.opcode) not in _ENGINE_OPS:
                continue
            if dep.engine != eng:
                continue
            inst.remove_dependency(dep_name)
            inst.add_dependency(dep_name, DI(sync=False, no_sync=True))
            n += 1
    return n



def build_nc(T=T_FULL):
    import concourse.mybir as mybir
    from concourse import bacc
    from concourse.masks import make_identity
    from concourse.tile import TileContext

    fp32 = mybir.dt.float32
    bf16 = mybir.dt.bfloat16
    Tanh = mybir.ActivationFunctionType.Tanh
    add = mybir.AluOpType.add
    mult = mybir.AluOpType.mult
    amax = mybir.AluOpType.max

    CT = 32 if T >= 32 else T   # x DMA chunk (timesteps)
    LA = 16 if T >= 32 else T   # transpose lookahead
    CPY = 8 if T >= 8 else T    # timesteps per PSUM->SBUF xT copy
    XT_RING = 32 if T >= 32 else T

    nc = bacc.Bacc(None, target_bir_lowering=False)

    x_d = nc.dram_tensor("x", [B, T, F], bf16, kind="ExternalInput")
    wA_d = nc.dram_tensor("wA", [F + 1, 4 * NR], bf16, kind="ExternalInput")
    wB_d = nc.dram_tensor("wB", [NR + 1, 4 * NR], bf16, kind="ExternalInput")
    wD_d = nc.dram_tensor("wD", [NR + 1, OUT], bf16, kind="ExternalInput")
    wG_d = nc.dram_tensor("wG", [NR + 1, H2], bf16, kind="ExternalInput")
    ri_d = nc.dram_tensor("rinit", [NR + 1, B], bf16, kind="ExternalInput")
    out_d = nc.dram_tensor("out", [OUT, B], fp32, kind="ExternalOutput")

    with TileContext(nc) as tc:
        with (
            tc.tile_pool(name="singles", bufs=1) as sp,
            tc.tile_pool(name="xraw", bufs=2) as xrp,
            tc.tile_pool(name="psum_z", bufs=2, space="PSUM") as pz,
            tc.tile_pool(name="psum_t", bufs=2, space="PSUM") as pt,
            tc.tile_pool(name="psum_zg", bufs=1, space="PSUM") as pzg,
            tc.tile_pool(name="psum_zo", bufs=1, space="PSUM") as pzo,
        ):
            wA = sp.tile([F + 1, 4 * NR], bf16)
            wB = sp.tile([NR + 1, 4 * NR], bf16)
            wD = sp.tile([NR + 1, OUT], bf16)
            wG = sp.tile([NR + 1, H2], bf16)
            # (weight DMAs are emitted after the warmup feed below so the
            # x-chunk DMA dispatches first — x gates the first transposes,
            # the weights aren't needed until the first matmul ~5us in)

            ident = sp.tile([64, 64], bf16)
            make_identity(nc, ident[:])

            # recurrent state [h1(0:32); h2(32:48); ones(48)], ping-pong
            rA = sp.tile([NR + 1, B], bf16)
            rB = sp.tile([NR + 1, B], bf16)
            rhs = [rA, rB]
            # zeros + ones row 48 (compute ops can't start at p48)
            nc.scalar.dma_start(rhs[0][:], ri_d[:])
            nc.gpsimd.dma_start(rhs[1][:], ri_d[:])

            # S cols: [Ti(0:64) | Tf(64:128) | Tg(128:192) | c-state(192:256)]
            # (gate blocks ordered [i|f|g|o] so [Tg | c] is contiguous)
            S = sp.tile([NR, 4 * B], bf16)
            STo = sp.tile([NR, B], bf16)     # tanh(z_o) both layers
            SPo = sp.tile([NR, B], bf16)     # sigma(o) both layers
            SP = sp.tile([NR, 2 * B], bf16)  # [sigma(i) | sigma(f)]
            M = sp.tile([NR, 2 * B], bf16)   # XY products
            # AUX: L2's XY source: [relu(g2)(0:64) | c2-state(64:128)]
            AUX = sp.tile([NR, 2 * B], bf16)
            TC = sp.tile([H1, B], bf16)      # tanh(c1')
            nc.gpsimd.memset(S[:, 3 * B : 4 * B], 0.0)  # c1 = 0
            nc.gpsimd.memset(AUX[:], 0.0)               # c2 = 0

            xT = sp.tile([F + 1, XT_RING * B], bf16)  # x^T ring + ones row
            nc.gpsimd.memset(xT[F : F + 1, :], 1.0)

            state = {"xraw": None, "psumT": None, "last_dve": None,
                     "last_pe": None}

            def _after(inst, anchor):
                # scheduler-only ordering: keep lookahead work out of the
                # critical chain by pinning it after this step's anchor op
                if anchor is not None:
                    import bass_rust

                    inst.ins.add_dependency(
                        anchor.ins.name,
                        bass_rust.DependencyInfo(sync=False, no_sync=True),
                    )

            def feed(k):
                t = k + LA
                if t >= T or t < 0:
                    return
                if t % CT == 0:
                    state["xraw"] = xrp.tile([B, CT * F], bf16, tag="xraw", name="xraw")
                    if t == 0:
                        # split the first chunk: the head lands ~4x sooner so
                        # the warmup transposes (and thus step 0) start early;
                        # weights dispatch on the ACT ring in between
                        nc.sync.dma_start(
                            state["xraw"][:, 0 : 8 * F], x_d[:, 0:8, :]
                        )
                        nc.scalar.dma_start(wA[:], wA_d[:])
                        nc.scalar.dma_start(wB[:], wB_d[:])
                        nc.sync.dma_start(
                            state["xraw"][:, 8 * F : CT * F], x_d[:, 8:CT, :]
                        )
                    else:
                        nc.sync.dma_start(state["xraw"][:], x_d[:, t : t + CT, :])
                if t % CPY == 0:
                    state["psumT"] = pt.tile(
                        [F, CPY * B], bf16, tag="psumT", name="psumT"
                    )
                j = t % CT
                tr = nc.tensor.transpose(
                    state["psumT"][:, (t % CPY) * B : (t % CPY + 1) * B],
                    state["xraw"][:, j * F : (j + 1) * F],
                    ident[:],
                )
                _after(tr, state["last_pe"])
                if t % CPY == CPY - 1:
                    base = (t - (CPY - 1)) % XT_RING
                    # PSUM->SBUF copy; pinned after this step's h' so it runs
                    # in the idle mm+ACT window, never mid-chain
                    cp = nc.vector.tensor_scalar_add(
                        xT[0:F, base * B : (base + CPY) * B], state["psumT"][:], 0.0
                    )
                    _after(cp, state["last_dve"])

            for k in range(-LA, 0):
                feed(k)

            # remaining prologue DMAs (wA/wB ride the ACT ring inside the
            # first feed above, right after the x head)
            nc.scalar.dma_start(wD[:], wD_d[:])
            nc.gpsimd.dma_start(wG[:], wG_d[:])

            for k in range(T + 1):
                r_cur = rhs[k % 2]
                r_nxt = rhs[(k + 1) % 2]
                l1 = k < T    # L1 computes h1_k
                l2 = k > 0    # L2 computes h2_{k-1}

                # Z holds [i|f|g] (the ACT_a chain read); the o block gets
                # its own tile/group so ACT_a's wait covers 3 recurrent
                # matmuls, not 4.
                Z = pz.tile([NR, 3 * B], fp32, tag="z", name="z")
                Zo = pzo.tile([NR, B], fp32, tag="zo", name="zo")
                if l1:
                    rk = k % XT_RING
                    for j in range(3):
                        nc.tensor.matmul(
                            Z[0:NR, j * B : (j + 1) * B],
                            wA[:, j * NR : (j + 1) * NR],
                            xT[:, rk * B : (rk + 1) * B],
                            start=(j == 0),
                            stop=False,
                        )
                    nc.tensor.matmul(
                        Zo[:],
                        wA[:, 3 * NR : 4 * NR],
                        xT[:, rk * B : (rk + 1) * B],
                        start=True,
                        stop=False,
                    )
                for j in range(3):
                    nc.tensor.matmul(
                        Z[0:NR, j * B : (j + 1) * B],
                        wB[:, j * NR : (j + 1) * NR],
                        r_cur[:],
                        start=(j == 0 and not l1),
                        stop=(j == 2),
                    )
                state["last_pe"] = nc.tensor.matmul(
                    Zo[:],
                    wB[:, 3 * NR : 4 * NR],
                    r_cur[:],
                    start=not l1,
                    stop=True,
                )

                if l2:
                    # g2 gets its own matmul + PSUM tile so its DVE relu
                    # does not share a PSUM bank with the ACT readers of Z
                    ZG = pzg.tile([H2, B], fp32, tag="zg", name="zg")
                    nc.tensor.matmul(ZG[:], wG[:], r_cur[:], start=True, stop=True)
                    nc.vector.tensor_scalar_max(AUX[H1:NR, 0:B], ZG[:], 0.0)
                # ACT: tanh over [i|f|g] (the chain op), then the o block
                nc.scalar.activation(S[:, 0 : 3 * B], Z[:], Tanh)
                nc.scalar.activation(STo[:], Zo[:], Tanh)

                # DVE cell phase
                # SP = [sigma(i) | sigma(f)] for both layers (4x ts)
                nc.vector.tensor_scalar(
                    SP[:], S[:, 0 : 2 * B], 0.5, 0.5, mult, add
                )
                if l1:
                    # [sigma(i1)*Tg1 | sigma(f1)*c1] — in1 = [Tg | c],
                    # contiguous cols 128:256
                    nc.vector.tensor_tensor(
                        M[0:H1, :], SP[0:H1, :], S[0:H1, 2 * B : 4 * B], mult
                    )
                    # c1' = i-term + f-term, back into the S tile
                    nc.vector.tensor_tensor(
                        S[0:H1, 3 * B : 4 * B], M[0:H1, 0:B], M[0:H1, B : 2 * B],
                        add,
                    )
                    # tanh(c1')
                    nc.scalar.activation(TC[:], S[0:H1, 3 * B : 4 * B], Tanh)
                # sigma(o) for both layers (off-chain: runs during TC)
                nc.vector.tensor_scalar(
                    SPo[:], STo[:], 0.5, 0.5, mult, add
                )
                if l2:
                    nc.vector.tensor_tensor(
                        M[H1:NR, :], SP[H1:NR, :], AUX[H1:NR, :], mult
                    )
                    nc.vector.tensor_tensor(
                        AUX[H1:NR, B : 2 * B], M[H1:NR, 0:B],
                        M[H1:NR, B : 2 * B], add,
                    )
                    # h2' = sigma(o2) * relu(c2') fused
                    state["last_dve"] = nc.vector.scalar_tensor_tensor(
                        r_nxt[H1:NR, :], AUX[H1:NR, B : 2 * B], 0.0,
                        SPo[H1:NR, :], amax, mult,
                    )
                if l1:
                    # h1' = sigma(o1)*tanh(c1') — last: waits on TC
                    state["last_dve"] = nc.vector.tensor_tensor(
                        r_nxt[0:H1, :], SPo[0:H1, :], TC[:], mult
                    )
                # feed at the BOTTOM of the iteration: the lookahead
                # transposes/copies then queue after this step's chain ops
                # and run in the idle mm+ACT window of the next step,
                # instead of being scheduler-placed mid-chain
                feed(k)

            # dense head: [0(h1) | Wd(h2) | bd]^T r_fin
            r_fin = rhs[(T + 1) % 2]
            opsum = pz.tile([OUT, B], fp32, tag="o", name="opsum")
            nc.tensor.matmul(opsum[:], wD[:], r_fin[:], start=True, stop=True)
            osb = sp.tile([OUT, B], fp32)
            nc.scalar.copy(osb[:], opsum[:])
            nc.sync.dma_start(out_d[:], osb[:])

            _downgrade_same_engine_syncs(nc)

    nc.compile()
    return nc


def _get_nc(T=T_FULL):
    if T not in _NC_CACHE:
        _NC_CACHE[T] = build_nc(T)
    return _NC_CACHE[T]


def _bf16(a):
    import ml_dtypes

    return np.asarray(a, np.float32).astype(ml_dtypes.bfloat16)


def prep_weights(W1, U1, b1, W2, U2, b2, Wd, bd):
    """Pack weights: gate blocks [g|i|f|o], sigma blocks (i,f,o) scaled 0.5
    (sigma(z) = 0.5*tanh(z/2)+0.5 with the /2 folded into the weights)."""
    W1, U1, b1 = (np.asarray(a, np.float32) for a in (W1, U1, b1))
    W2, U2, b2 = (np.asarray(a, np.float32) for a in (W2, U2, b2))
    Wd, bd = np.asarray(Wd, np.float32), np.asarray(bd, np.float32)

    def blocks(w, H):  # reference order i,f,g,o -> [i, f, g, o]
        return [w[..., j * H : (j + 1) * H] for j in range(4)]

    cs = [0.5, 0.5, 1.0, 0.5]  # z scale per block [i|f|g|o]

    W1b, b1b = blocks(W1, H1), blocks(b1, H1)
    U1b = blocks(U1, H1)
    W2b, U2b, b2b = blocks(W2, H2), blocks(U2, H2), blocks(b2, H2)

    # wA[j]: [65, 48] = [[W1_j*s; b1_j*s] | zeros(L2)]
    wA = np.concatenate(
        [
            np.concatenate(
                [
                    np.concatenate([W1b[j] * cs[j], b1b[j][None, :] * cs[j]], axis=0),
                    np.zeros((F + 1, H2), np.float32),
                ],
                axis=1,
            )
            for j in range(4)
        ],
        axis=1,
    )
    # wB[j]: [49, 48] = [[U1_j*s; 0] | [W2_j*s; U2_j*s; b2_j*s]]
    wB = np.concatenate(
        [
            np.concatenate(
                [
                    np.concatenate(
                        [U1b[j] * cs[j], np.zeros((H2 + 1, H1), np.float32)], axis=0
                    ),
                    np.concatenate(
                        [W2b[j] * cs[j], U2b[j] * cs[j], b2b[j][None, :] * cs[j]],
                        axis=0,
                    ),
                ],
                axis=1,
            )
            for j in range(4)
        ],
        axis=1,
    )
    wD = np.concatenate(
        [np.zeros((H1, OUT), np.float32), Wd, bd[None, :]], axis=0
    )
    # wG [49, 16]: L2's raw g2 pre-activation [W2_g; U2_g; b2_g]
    wG = np.concatenate(
        [W2b[2], U2b[2], b2b[2][None, :]], axis=0
    )
    return _bf16(wA), _bf16(wB), _bf16(wD), _bf16(wG)


def make_rinit():
    rinit = np.zeros((NR + 1, B), np.float32)
    rinit[NR, :] = 1.0
    return _bf16(rinit)


def run_cores(nc, x, weights, T, trace=False):
    from concourse.bass_utils import run_bass_kernel_spmd

    x = _bf16(x)
    weights = dict(weights, rinit=make_rinit())
    in_maps = [
        dict(x=np.ascontiguousarray(x[c * B : (c + 1) * B, :T]), **weights)
        for c in range(N_CORES)
    ]
    res = run_bass_kernel_spmd(nc, in_maps, core_ids=list(range(N_CORES)), trace=trace)
    out = np.concatenate([np.asarray(r["out"]).T for r in res.results], axis=0)
    return out.astype(np.float32), res


def kernel(x, W1, U1, b1, W2, U2, b2, Wd, bd):
    wA, wB, wD, wG = prep_weights(W1, U1, b1, W2, U2, b2, Wd, bd)
    nc = _get_nc(T_FULL)
    out, _ = run_cores(nc, x, dict(wA=wA, wB=wB, wD=wD, wG=wG), T_FULL)
    return out


# revision 13
# speedup vs baseline: 1.6491x; 1.0000x over previous
"""Trainium2 Bass kernel for a 2-layer LSTM + Dense head — v4 (wide hybrid).

Model: L1 LSTM(32, tanh), L2 LSTM(16, relu), Dense(12) on last h2.
x: [512, 512, 64] f32. Data-parallel: batch 512 -> 64 per core, 8 cores.

Layout: both layers merged on partitions (L1 rows 0:32, L2 rows 32:48),
gates along the free dim in blocks [g|i|f|o] x 64 batch — so every
two-tensor DVE op pairs operands at the SAME partition base (a hard
walrus/birverifier requirement for SBUF operands).

Key optimizations over the original baseline:
- Single activation function: sigma(x) = 0.5*tanh(x/2)+0.5 with the /2
  pre-folded into the i,f,o weight columns, so ONE tanh op covers the
  [g|i|f] blocks of BOTH layers (the o block gets its own op, off the
  critical chain: sigma(o) is only needed after the c' tanh).
- The c-state lives in the S tile's last column block (192:256) right
  after Tg (128:192), so the XY product reads [Tg | c] as one contiguous
  AP and computes [sigma(i)*tau(g) | sigma(f)*c] in a single tensor_tensor.
- relu(g2) + c2 live in AUX so L2's XY mirrors L1's, and
  h2' = sigma(o2)*relu(c2') is one fused scalar_tensor_tensor (max,mult).
- bf16 matmuls (1 PE cycle/row vs 4 for fp32) and bf16 DVE (2x/4x modes).
- Same-engine sync deps downgraded to queue-order deps (depfix).
Measured (TimelineSim, T=512): 1.012ms total vs baseline 1.629ms (1.61x);
rel err 1.04e-2 on the full axon/walrus path.
"""

import sys

import numpy as np

if "/opt/trn_rl_repo" not in sys.path:
    sys.path.insert(0, "/opt/trn_rl_repo")

B_FULL = 512
T_FULL = 512
F = 64
H1, H2, OUT = 32, 16, 12
N_CORES = 8
B = B_FULL // N_CORES  # 64 batch per core

NR = H1 + H2  # 48 state rows [h1 | h2]
GB = NR       # gate block width (cols per gate block)

_NC_CACHE = {}


# Opcodes that execute synchronously on the issuing engine's pipeline.
_ENGINE_OPS = {
    "Matmult", "Activation", "TensorTensor", "TensorScalarPtr",
    "TensorScalar", "TensorReduce", "TensorTensorScan", "Select",
    "Iota", "Memset", "LoadActFuncSet", "Ldweights",
}


def _downgrade_same_engine_syncs(nc):
    """Engines execute their queue in order and interlock their own
    pipelines (the DVE DRAIN, not the semaphore, is the ordering barrier),
    so a same-engine data dependency needs no semaphore wait. Downgrading
    those edges to ordering-only removes ~95ns (DVE) / ~219ns (ACT) of
    serial latency per hop. Cross-engine and DMA edges are untouched."""
    import bass_rust

    DI = bass_rust.DependencyInfo
    inst_map = nc.inst_map
    n = 0
    for inst in list(inst_map.values()):
        if str(inst.opcode) not in _ENGINE_OPS:
            continue
        eng = inst.engine
        if eng is None:
            continue
        for dep_name, info in list(inst.dependency_edges()):
            if not info.sync:
                continue
            dep = inst_map.get(dep_name)
            if dep is None or str(dep.opcode) not in _ENGINE_OPS:
                continue
            if dep.engine != eng:
                continue
            inst.remove_dependency(dep_name)
            inst.add_dependency(dep_name, DI(sync=False, no_sync=True))
            n += 1
    return n



def build_nc(T=T_FULL):
    import concourse.mybir as mybir
    from concourse import bacc
    from concourse.masks import make_identity
    from concourse.tile import TileContext

    fp32 = mybir.dt.float32
    bf16 = mybir.dt.bfloat16
    Tanh = mybir.ActivationFunctionType.Tanh
    add = mybir.AluOpType.add
    mult = mybir.AluOpType.mult
    amax = mybir.AluOpType.max

    CT = 32 if T >= 32 else T   # x DMA chunk (timesteps)
    LA = 16 if T >= 32 else T   # transpose lookahead
    CPY = 8 if T >= 8 else T    # timesteps per PSUM->SBUF xT copy
    XT_RING = 32 if T >= 32 else T

    nc = bacc.Bacc(None, target_bir_lowering=False)

    x_d = nc.dram_tensor("x", [B, T, F], bf16, kind="ExternalInput")
    wA_d = nc.dram_tensor("wA", [F + 1, 4 * NR], bf16, kind="ExternalInput")
    wB_d = nc.dram_tensor("wB", [NR + 1, 4 * NR], bf16, kind="ExternalInput")
    wD_d = nc.dram_tensor("wD", [NR + 1, OUT], bf16, kind="ExternalInput")
    wG_d = nc.dram_tensor("wG", [NR + 1, H2], bf16, kind="ExternalInput")
    ri_d = nc.dram_tensor("rinit", [NR + 1, B], bf16, kind="ExternalInput")
    out_d = nc.dram_tensor("out", [OUT, B], fp32, kind="ExternalOutput")

    with TileContext(nc) as tc:
        with (
            tc.tile_pool(name="singles", bufs=1) as sp,
            tc.tile_pool(name="xraw", bufs=2) as xrp,
            tc.tile_pool(name="psum_z", bufs=2, space="PSUM") as pz,
            tc.tile_pool(name="psum_t", bufs=2, space="PSUM") as pt,
            tc.tile_pool(name="psum_zg", bufs=1, space="PSUM") as pzg,
            tc.tile_pool(name="psum_zo", bufs=1, space="PSUM") as pzo,
        ):
            wA = sp.tile([F + 1, 4 * NR], bf16)
            wB = sp.tile([NR + 1, 4 * NR], bf16)
            wD = sp.tile([NR + 1, OUT], bf16)
            wG = sp.tile([NR + 1, H2], bf16)
            # (weight DMAs are emitted after the warmup feed below so the
            # x-chunk DMA dispatches first — x gates the first transposes,
            # the weights aren't needed until the first matmul ~5us in)

            ident = sp.tile([64, 64], bf16)
            make_identity(nc, ident[:])

            # recurrent state [h1(0:32); h2(32:48); ones(48)], ping-pong
            rA = sp.tile([NR + 1, B], bf16)
            rB = sp.tile([NR + 1, B], bf16)
            rhs = [rA, rB]
            # zeros + ones row 48 (compute ops can't start at p48)
            nc.scalar.dma_start(rhs[0][:], ri_d[:])
            nc.gpsimd.dma_start(rhs[1][:], ri_d[:])

            # S cols: [Ti(0:64) | Tf(64:128) | Tg(128:192) | c-state(192:256)]
            # (gate blocks ordered [i|f|g|o] so [Tg | c] is contiguous)
            S = sp.tile([NR, 4 * B], bf16)
            STo = sp.tile([NR, B], bf16)     # tanh(z_o) both layers
            SPo = sp.tile([NR, B], bf16)     # sigma(o) both layers
            SP = sp.tile([NR, 2 * B], bf16)  # [sigma(i) | sigma(f)]
            M = sp.tile([NR, 2 * B], bf16)   # XY products
            # AUX: L2's XY source: [relu(g2)(0:64) | c2-state(64:128)]
            AUX = sp.tile([NR, 2 * B], bf16)
            TC = sp.tile([H1, B], bf16)      # tanh(c1')
            nc.gpsimd.memset(S[:, 3 * B : 4 * B], 0.0)  # c1 = 0
            nc.gpsimd.memset(AUX[:], 0.0)               # c2 = 0

            xT = sp.tile([F + 1, XT_RING * B], bf16)  # x^T ring + ones row
            nc.gpsimd.memset(xT[F : F + 1, :], 1.0)

            state = {"xraw": None, "psumT": None, "last_dve": None,
                     "last_pe": None}

            def _after(inst, anchor):
                # scheduler-only ordering: keep lookahead work out of the
                # critical chain by pinning it after this step's anchor op
                if anchor is not None:
                    import bass_rust

                    inst.ins.add_dependency(
                        anchor.ins.name,
                        bass_rust.DependencyInfo(sync=False, no_sync=True),
                    )

            def feed(k):
                t = k + LA
                if t >= T or t < 0:
                    return
                if t % CT == 0:
                    state["xraw"] = xrp.tile([B, CT * F], bf16, tag="xraw", name="xraw")
                    if t == 0:
                        # split the first chunk: the head lands ~4x sooner so
                        # the warmup transposes (and thus step 0) start early;
                        # weights dispatch on the ACT ring in between
                        nc.sync.dma_start(
                            state["xraw"][:, 0 : 8 * F], x_d[:, 0:8, :]
                        )
                        nc.scalar.dma_start(wA[:], wA_d[:])
                        nc.scalar.dma_start(wB[:], wB_d[:])
                        nc.sync.dma_start(
                            state["xraw"][:, 8 * F : CT * F], x_d[:, 8:CT, :]
                        )
                    else:
                        nc.sync.dma_start(state["xraw"][:], x_d[:, t : t + CT, :])
                if t % CPY == 0:
                    state["psumT"] = pt.tile(
                        [F, CPY * B], bf16, tag="psumT", name="psumT"
                    )
                j = t % CT
                tr = nc.tensor.transpose(
                    state["psumT"][:, (t % CPY) * B : (t % CPY + 1) * B],
                    state["xraw"][:, j * F : (j + 1) * F],
                    ident[:],
                )
                _after(tr, state["last_pe"])
                if t % CPY == CPY - 1:
                    base = (t - (CPY - 1)) % XT_RING
                    # PSUM->SBUF copy; pinned after this step's h' so it runs
                    # in the idle mm+ACT window, never mid-chain
                    cp = nc.vector.tensor_scalar_add(
                        xT[0:F, base * B : (base + CPY) * B], state["psumT"][:], 0.0
                    )
                    _after(cp, state["last_dve"])

            # warmup: only the first CPY timesteps — step 0 must not queue
            # behind transposes that wait for the x-tail DMA; the rest
            # catch up at the bottom of iteration 0
            for k in range(-LA, -LA + CPY):
                feed(k)

            # remaining prologue DMAs (wA/wB ride the ACT ring inside the
            # first feed above, right after the x head)
            nc.scalar.dma_start(wD[:], wD_d[:])
            nc.gpsimd.dma_start(wG[:], wG_d[:])

            for k in range(T + 1):
                r_cur = rhs[k % 2]
                r_nxt = rhs[(k + 1) % 2]
                l1 = k < T    # L1 computes h1_k
                l2 = k > 0    # L2 computes h2_{k-1}

                # Z holds [i|f|g] (the ACT_a chain read); the o block gets
                # its own tile/group so ACT_a's wait covers 3 recurrent
                # matmuls, not 4.
                Z = pz.tile([NR, 3 * B], fp32, tag="z", name="z")
                Zo = pzo.tile([NR, B], fp32, tag="zo", name="zo")
                if l1:
                    rk = k % XT_RING
                    for j in range(3):
                        nc.tensor.matmul(
                            Z[0:NR, j * B : (j + 1) * B],
                            wA[:, j * NR : (j + 1) * NR],
                            xT[:, rk * B : (rk + 1) * B],
                            start=(j == 0),
                            stop=False,
                        )
                    nc.tensor.matmul(
                        Zo[:],
                        wA[:, 3 * NR : 4 * NR],
                        xT[:, rk * B : (rk + 1) * B],
                        start=True,
                        stop=False,
                    )
                for j in range(3):
                    nc.tensor.matmul(
                        Z[0:NR, j * B : (j + 1) * B],
                        wB[:, j * NR : (j + 1) * NR],
                        r_cur[:],
                        start=(j == 0 and not l1),
                        stop=(j == 2),
                    )
                state["last_pe"] = nc.tensor.matmul(
                    Zo[:],
                    wB[:, 3 * NR : 4 * NR],
                    r_cur[:],
                    start=not l1,
                    stop=True,
                )

                if l2:
                    # g2 gets its own matmul + PSUM tile so its DVE relu
                    # does not share a PSUM bank with the ACT readers of Z
                    ZG = pzg.tile([H2, B], fp32, tag="zg", name="zg")
                    nc.tensor.matmul(ZG[:], wG[:], r_cur[:], start=True, stop=True)
                    nc.vector.tensor_scalar_max(AUX[H1:NR, 0:B], ZG[:], 0.0)
                # ACT: tanh over [i|f|g] (the chain op), then the o block
                nc.scalar.activation(S[:, 0 : 3 * B], Z[:], Tanh)
                nc.scalar.activation(STo[:], Zo[:], Tanh)

                # DVE cell phase
                # SP = [sigma(i) | sigma(f)] for both layers (4x ts)
                nc.vector.tensor_scalar(
                    SP[:], S[:, 0 : 2 * B], 0.5, 0.5, mult, add
                )
                if l1:
                    # [sigma(i1)*Tg1 | sigma(f1)*c1] — in1 = [Tg | c],
                    # contiguous cols 128:256
                    nc.vector.tensor_tensor(
                        M[0:H1, :], SP[0:H1, :], S[0:H1, 2 * B : 4 * B], mult
                    )
                    # c1' = i-term + f-term, back into the S tile
                    nc.vector.tensor_tensor(
                        S[0:H1, 3 * B : 4 * B], M[0:H1, 0:B], M[0:H1, B : 2 * B],
                        add,
                    )
                    # tanh(c1')
                    nc.scalar.activation(TC[:], S[0:H1, 3 * B : 4 * B], Tanh)
                # sigma(o) for both layers (off-chain: runs during TC)
                nc.vector.tensor_scalar(
                    SPo[:], STo[:], 0.5, 0.5, mult, add
                )
                if l2:
                    nc.vector.tensor_tensor(
                        M[H1:NR, :], SP[H1:NR, :], AUX[H1:NR, :], mult
                    )
                    nc.vector.tensor_tensor(
                        AUX[H1:NR, B : 2 * B], M[H1:NR, 0:B],
                        M[H1:NR, B : 2 * B], add,
                    )
                    # h2' = sigma(o2) * relu(c2') fused
                    state["last_dve"] = nc.vector.scalar_tensor_tensor(
                        r_nxt[H1:NR, :], AUX[H1:NR, B : 2 * B], 0.0,
                        SPo[H1:NR, :], amax, mult,
                    )
                if l1:
                    # h1' = sigma(o1)*tanh(c1') — last: waits on TC
                    state["last_dve"] = nc.vector.tensor_tensor(
                        r_nxt[0:H1, :], SPo[0:H1, :], TC[:], mult
                    )
                # feed at the BOTTOM of the iteration: the lookahead
                # transposes/copies then queue after this step's chain ops
                # and run in the idle mm+ACT window of the next step,
                # instead of being scheduler-placed mid-chain
                if k == 0:
                    for kk in range(-LA + CPY, 0):  # warmup catch-up
                        feed(kk)
                feed(k)

            # dense head: [0(h1) | Wd(h2) | bd]^T r_fin
            r_fin = rhs[(T + 1) % 2]
            opsum = pz.tile([OUT, B], fp32, tag="o", name="opsum")
            nc.tensor.matmul(opsum[:], wD[:], r_fin[:], start=True, stop=True)
            osb = sp.tile([OUT, B], fp32)
            nc.scalar.copy(osb[:], opsum[:])
            nc.sync.dma_start(out_d[:], osb[:])

            _downgrade_same_engine_syncs(nc)

    nc.compile()
    return nc


def _get_nc(T=T_FULL):
    if T not in _NC_CACHE:
        _NC_CACHE[T] = build_nc(T)
    return _NC_CACHE[T]


def _bf16(a):
    import ml_dtypes

    return np.asarray(a, np.float32).astype(ml_dtypes.bfloat16)


def prep_weights(W1, U1, b1, W2, U2, b2, Wd, bd):
    """Pack weights: gate blocks [g|i|f|o], sigma blocks (i,f,o) scaled 0.5
    (sigma(z) = 0.5*tanh(z/2)+0.5 with the /2 folded into the weights)."""
    W1, U1, b1 = (np.asarray(a, np.float32) for a in (W1, U1, b1))
    W2, U2, b2 = (np.asarray(a, np.float32) for a in (W2, U2, b2))
    Wd, bd = np.asarray(Wd, np.float32), np.asarray(bd, np.float32)

    def blocks(w, H):  # reference order i,f,g,o -> [i, f, g, o]
        return [w[..., j * H : (j + 1) * H] for j in range(4)]

    cs = [0.5, 0.5, 1.0, 0.5]  # z scale per block [i|f|g|o]

    W1b, b1b = blocks(W1, H1), blocks(b1, H1)
    U1b = blocks(U1, H1)
    W2b, U2b, b2b = blocks(W2, H2), blocks(U2, H2), blocks(b2, H2)

    # wA[j]: [65, 48] = [[W1_j*s; b1_j*s] | zeros(L2)]
    wA = np.concatenate(
        [
            np.concatenate(
                [
                    np.concatenate([W1b[j] * cs[j], b1b[j][None, :] * cs[j]], axis=0),
                    np.zeros((F + 1, H2), np.float32),
                ],
                axis=1,
            )
            for j in range(4)
        ],
        axis=1,
    )
    # wB[j]: [49, 48] = [[U1_j*s; 0] | [W2_j*s; U2_j*s; b2_j*s]]
    wB = np.concatenate(
        [
            np.concatenate(
                [
                    np.concatenate(
                        [U1b[j] * cs[j], np.zeros((H2 + 1, H1), np.float32)], axis=0
                    ),
                    np.concatenate(
                        [W2b[j] * cs[j], U2b[j] * cs[j], b2b[j][None, :] * cs[j]],
                        axis=0,
                    ),
                ],
                axis=1,
            )
            for j in range(4)
        ],
        axis=1,
    )
    wD = np.concatenate(
        [np.zeros((H1, OUT), np.float32), Wd, bd[None, :]], axis=0
    )
    # wG [49, 16]: L2's raw g2 pre-activation [W2_g; U2_g; b2_g]
    wG = np.concatenate(
        [W2b[2], U2b[2], b2b[2][None, :]], axis=0
    )
    return _bf16(wA), _bf16(wB), _bf16(wD), _bf16(wG)


def make_rinit():
    rinit = np.zeros((NR + 1, B), np.float32)
    rinit[NR, :] = 1.0
    return _bf16(rinit)


def run_cores(nc, x, weights, T, trace=False):
    from concourse.bass_utils import run_bass_kernel_spmd

    x = _bf16(x)
    weights = dict(weights, rinit=make_rinit())
    in_maps = [
        dict(x=np.ascontiguousarray(x[c * B : (c + 1) * B, :T]), **weights)
        for c in range(N_CORES)
    ]
    res = run_bass_kernel_spmd(nc, in_maps, core_ids=list(range(N_CORES)), trace=trace)
    out = np.concatenate([np.asarray(r["out"]).T for r in res.results], axis=0)
    return out.astype(np.float32), res


def kernel(x, W1, U1, b1, W2, U2, b2, Wd, bd):
    wA, wB, wD, wG = prep_weights(W1, U1, b1, W2, U2, b2, Wd, bd)
    nc = _get_nc(T_FULL)
    out, _ = run_cores(nc, x, dict(wA=wA, wB=wB, wD=wD, wG=wG), T_FULL)
    return out
